# revision 12
# baseline (speedup 1.0000x reference)
"""Trainium2 Bass kernel for nn_Mirror: per-sample conditional flips + fp16 cast.

Full op: x [16,2,64,128,128] f32, x_flag [16], y_flag [16] f32 ->
out [16,2,64,128,128] f16 where per sample b:
  out[b] = 0                 if x_flag[b] <= 0.5
         = flip_h(x[b])      if x_flag[b] > 0.5 and y_flag[b] <= 0.5
         = flip_hw(x[b])     if x_flag[b] > 0.5 and y_flag[b] > 0.5

Device kernel (per core, 2 sample slots):
  A sample [2,64,128,128] is 128 images of 128x128 -> images map to the 128
  SBUF partitions, so both flips are free-dim manipulations.  Per 2048-elem
  free chunk j of a sample:
    load  T <- x[b] chunk j            (sync HWDGE, cond = x_flag[b] > 0.5)
    O = revh(T) cast fp16              (single 1-input pass, DVE/ACT alternate)
    store out[b] chunk j     <- O      (gpsimd SWDGE, cond = active & !yflip)
    store out[b] chunk 7-j   <- revw(O)(gpsimd SWDGE, cond = active &  yflip;
                                        w-reversal on the SBUF-side read AP)
  Flag compares run on raw float bits in engine registers (signed-int compare
  against bits(0.5f) matches float > 0.5 for non-NaN inputs).  Skipped DMAs
  still bump their semaphores, so Tile's schedule is oblivious to the flags.
  Inactive samples move zero bytes; output DRAM is pre-zeroed by the runtime.

Host scheduling: the flags are host-visible, so active samples are assigned
round-robin to (core, slot) across the 8 cores; inactive samples never ship
and their output stays host-side zeros.  With k active samples the busiest
core processes ceil(k/8) samples, i.e. half traffic whenever k <= 8.
"""

import numpy as np

import concourse.bass as bass
import concourse.mybir as mybir
import concourse.tile as tile
from concourse import bacc
from concourse.bass_utils import run_bass_kernel_spmd
from concourse.expressions import s_logical_and
from concourse.ordered_set import OrderedSet

N_CORES = 8
FULL_B = 16
B = 2                  # sample slots per core
C, D, W, H = 2, 64, 128, 128
WH = W * H             # 16384 free elems per image
CH = 2048              # free-chunk size (16 w-rows)
NCH = WH // CH         # 8 chunks per sample
F_HALF = 0x3F000000    # bits of 0.5f

SP = mybir.EngineType.SP
POOL = mybir.EngineType.Pool


def build_program(sim_init=False):
    nc = bacc.Bacc("TRN2", target_bir_lowering=False, debug=False)
    x = nc.dram_tensor("x", [B, C, D, W, H], mybir.dt.float32, kind="ExternalInput")
    xf = nc.dram_tensor("x_flag", [B], mybir.dt.float32, kind="ExternalInput")
    yf = nc.dram_tensor("y_flag", [B], mybir.dt.float32, kind="ExternalInput")
    out = nc.dram_tensor(
        "out", [B, C, D, W, H], mybir.dt.float16, kind="ExternalOutput"
    )

    xv = x.ap().rearrange("b c d w h -> b (c d) (w h)")  # [B, 128, 16384]
    ov = out.ap().rearrange("b c d w h -> b (c d) (w h)")

    with tile.TileContext(nc) as tc:
        with (
            tc.tile_pool(name="flags", bufs=1) as flag_pool,
            tc.tile_pool(name="in", bufs=12) as in_pool,
            tc.tile_pool(name="out", bufs=10) as out_pool,
        ):
            fx = flag_pool.tile([1, B], mybir.dt.float32, tag="fx")
            fy = flag_pool.tile([1, B], mybir.dt.float32, tag="fy")
            nc.sync.dma_start(fx[:], xf.ap().unsqueeze(0))
            nc.sync.dma_start(fy[:], yf.ap().unsqueeze(0))

            # per-sample flag bits in registers on the DMA-issuing engines
            conds = []
            for b in range(B):
                xr = nc.alloc_registers(f"xr{b}", engines=OrderedSet([SP, POOL]))
                yr = nc.alloc_registers(f"yr{b}", engines=OrderedSet([POOL]))
                nc.regs_load(xr, fx[0:1, b : b + 1].bitcast(mybir.dt.int32))
                nc.regs_load(yr, fy[0:1, b : b + 1].bitcast(mybir.dt.int32))
                act_sp = nc.snap(xr, engines=OrderedSet([SP])) > F_HALF
                xg = nc.snap(xr, engines=OrderedSet([POOL])) > F_HALF
                yg = nc.snap(yr, engines=OrderedSet([POOL])) > F_HALF
                yl = nc.snap(yr, engines=OrderedSet([POOL])) <= F_HALF
                conds.append(
                    (act_sp, s_logical_and(xg, yl), s_logical_and(xg, yg))
                )

            n = 0
            for b in range(B):
                act_sp, c_a, c_b = conds[b]
                for j in range(NCH):
                    jb = NCH - 1 - j
                    t = in_pool.tile([128, CH], mybir.dt.float32, tag="tin")
                    if sim_init:
                        # CoreSim-only: skipped loads leave tiles uninit,
                        # which the sim rejects; HW reads garbage that is
                        # never stored.
                        nc.gpsimd.memset(t[:], 0.0)
                    nc.sync.dma_start(
                        t[:], xv[b, :, j * CH : (j + 1) * CH], cond=act_sp
                    )
                    og = out_pool.tile([128, CH], mybir.dt.float16, tag="og")
                    # O = revh(T), cast to fp16; single 1-input DVE pass
                    # (DVE CAST beats ACT COPY and avoids ACT table/drains)
                    src = t[:].rearrange("p (w h) -> p w h", h=H)[:, :, ::-1]
                    dst = og[:].rearrange("p (w h) -> p w h", h=H)
                    nc.vector.tensor_copy(dst, src)
                    n += 1
                    # store A: no w-flip
                    nc.gpsimd.dma_start(
                        ov[b, :, j * CH : (j + 1) * CH], og[:], cond=c_a
                    )
                    # store B: w-flip via reversed w-block read of O
                    og_wrev = og[:].rearrange("p (w h) -> p w h", h=H)[:, ::-1, :]
                    nc.gpsimd.dma_start(
                        ov[b, :, jb * CH : (jb + 1) * CH], og_wrev, cond=c_b
                    )
    nc.compile()
    return nc


_NC_CACHE = None


def _get_program():
    global _NC_CACHE
    if _NC_CACHE is None:
        _NC_CACHE = build_program()
    return _NC_CACHE


def kernel(x, x_flag, y_flag, _trace=False, **trace_kwargs):
    x = np.asarray(x)
    if x.dtype != np.float32:
        x = x.astype(np.float32)
    x_flag = np.asarray(x_flag, dtype=np.float32)
    y_flag = np.asarray(y_flag, dtype=np.float32)
    n = x.shape[0]
    assert n == FULL_B, x.shape

    # host-side schedule: only active samples do device work; spread them
    # round-robin over cores so the busiest core gets ceil(k / n_cores)
    active = [int(i) for i in np.nonzero(x_flag > 0.5)[0]]
    # (core, slot) -> sample index
    assign = {}
    for i, idx in enumerate(active):
        assign[(i % N_CORES, i // N_CORES)] = idx
    assert len(active) <= N_CORES * B

    sample_shape = x.shape[1:]
    dummy = np.zeros((1,) + sample_shape, dtype=np.float32)
    in_maps = []
    for c in range(N_CORES):
        xs, xfs, yfs = [], [], []
        for s in range(B):
            idx = assign.get((c, s))
            if idx is None:
                xs.append(dummy[0])
                xfs.append(0.0)
                yfs.append(0.0)
            else:
                xs.append(x[idx])
                xfs.append(float(x_flag[idx]))
                yfs.append(float(y_flag[idx]))
        in_maps.append(
            {
                "x": np.stack(xs),
                "x_flag": np.array(xfs, dtype=np.float32),
                "y_flag": np.array(yfs, dtype=np.float32),
            }
        )

    nc = _get_program()
    res = run_bass_kernel_spmd(
        nc, in_maps, core_ids=list(range(N_CORES)), trace=_trace, **trace_kwargs
    )

    out = np.zeros((FULL_B,) + sample_shape, dtype=np.float16)
    for (c, s), idx in assign.items():
        out[idx] = res.results[c]["out"][s]
    if _trace:
        return out, res
    return out


# revision 15
# speedup vs baseline: 1.3472x; 1.3472x over previous
"""Trainium2 Bass kernel for nn_Mirror: per-sample conditional flips + fp16 cast.

Full op: x [16,2,64,128,128] f32, x_flag [16], y_flag [16] f32 ->
out [16,2,64,128,128] f16 where per sample b:
  out[b] = 0                 if x_flag[b] <= 0.5
         = flip_h(x[b])      if x_flag[b] > 0.5 and y_flag[b] <= 0.5
         = flip_hw(x[b])     if x_flag[b] > 0.5 and y_flag[b] > 0.5

Device kernel (per core, 2 sample slots):
  A sample [2,64,128,128] is 128 images of 128x128 -> images map to the 128
  SBUF partitions, so both flips are free-dim manipulations.  Per 2048-elem
  free chunk j of a sample:
    load  T <- x[b] chunk j            (sync HWDGE, cond = x_flag[b] > 0.5)
    O = revh(T) cast fp16              (single 1-input pass, DVE/ACT alternate)
    store out[b] chunk j     <- O      (gpsimd SWDGE, cond = active & !yflip)
    store out[b] chunk 7-j   <- revw(O)(gpsimd SWDGE, cond = active &  yflip;
                                        w-reversal on the SBUF-side read AP)
  Flag compares run on raw float bits in engine registers (signed-int compare
  against bits(0.5f) matches float > 0.5 for non-NaN inputs).  Skipped DMAs
  still bump their semaphores, so Tile's schedule is oblivious to the flags.
  Inactive samples move zero bytes; output DRAM is pre-zeroed by the runtime.

Host scheduling: the flags are host-visible, so active samples are assigned
round-robin to (core, slot) across the 8 cores; inactive samples never ship
and their output stays host-side zeros.  With k active samples the busiest
core processes ceil(k/8) samples, i.e. half traffic whenever k <= 8.
"""

import numpy as np

import concourse.bass as bass
import concourse.mybir as mybir
import concourse.tile as tile
from concourse import bacc
from concourse.bass_utils import run_bass_kernel_spmd
from concourse.expressions import s_logical_and
from concourse.ordered_set import OrderedSet

N_CORES = 8
FULL_B = 16
B = 2                  # sample slots per core
C, D, W, H = 2, 64, 128, 128
WH = W * H             # 16384 free elems per image
CH = 4096              # free-chunk size (32 w-rows)
NCH = WH // CH         # 4 chunks per sample
F_HALF = 0x3F000000    # bits of 0.5f

SP = mybir.EngineType.SP
POOL = mybir.EngineType.Pool


def build_program(sim_init=False):
    nc = bacc.Bacc("TRN2", target_bir_lowering=False, debug=False)
    x = nc.dram_tensor("x", [B, C, D, W, H], mybir.dt.float32, kind="ExternalInput")
    xf = nc.dram_tensor("x_flag", [B], mybir.dt.float32, kind="ExternalInput")
    yf = nc.dram_tensor("y_flag", [B], mybir.dt.float32, kind="ExternalInput")
    out = nc.dram_tensor(
        "out", [B, C, D, W, H], mybir.dt.float16, kind="ExternalOutput"
    )

    xv = x.ap().rearrange("b c d w h -> b (c d) (w h)")  # [B, 128, 16384]
    ov = out.ap().rearrange("b c d w h -> b (c d) (w h)")

    with tile.TileContext(nc) as tc:
        with (
            tc.tile_pool(name="flags", bufs=1) as flag_pool,
            tc.tile_pool(name="in", bufs=7) as in_pool,
            tc.tile_pool(name="out", bufs=6) as out_pool,
        ):
            fx = flag_pool.tile([1, B], mybir.dt.float32, tag="fx")
            fy = flag_pool.tile([1, B], mybir.dt.float32, tag="fy")
            nc.sync.dma_start(fx[:], xf.ap().unsqueeze(0))
            nc.sync.dma_start(fy[:], yf.ap().unsqueeze(0))

            # per-sample flag bits in registers on the DMA-issuing engines
            conds = []
            for b in range(B):
                xr = nc.alloc_registers(f"xr{b}", engines=OrderedSet([SP, POOL]))
                yr = nc.alloc_registers(f"yr{b}", engines=OrderedSet([POOL]))
                nc.regs_load(xr, fx[0:1, b : b + 1].bitcast(mybir.dt.int32))
                nc.regs_load(yr, fy[0:1, b : b + 1].bitcast(mybir.dt.int32))
                act_sp = nc.snap(xr, engines=OrderedSet([SP])) > F_HALF
                xg = nc.snap(xr, engines=OrderedSet([POOL])) > F_HALF
                yg = nc.snap(yr, engines=OrderedSet([POOL])) > F_HALF
                yl = nc.snap(yr, engines=OrderedSet([POOL])) <= F_HALF
                conds.append(
                    (act_sp, s_logical_and(xg, yl), s_logical_and(xg, yg))
                )

            n = 0
            # interleave the two sample slots so an inactive slot's dead
            # compute overlaps the active slot's HBM-bound stream instead
            # of padding the kernel tail
            for j in range(NCH):
                for b in range(B):
                    act_sp, c_a, c_b = conds[b]
                    jb = NCH - 1 - j
                    t = in_pool.tile([128, CH], mybir.dt.float32, tag="tin")
                    if sim_init:
                        # CoreSim-only: skipped loads leave tiles uninit,
                        # which the sim rejects; HW reads garbage that is
                        # never stored.
                        nc.gpsimd.memset(t[:], 0.0)
                    nc.sync.dma_start(
                        t[:], xv[b, :, j * CH : (j + 1) * CH], cond=act_sp
                    )
                    og = out_pool.tile([128, CH], mybir.dt.float16, tag="og")
                    # O = revh(T), cast to fp16; single 1-input DVE pass
                    # (DVE CAST beats ACT COPY and avoids ACT table/drains)
                    src = t[:].rearrange("p (w h) -> p w h", h=H)[:, :, ::-1]
                    dst = og[:].rearrange("p (w h) -> p w h", h=H)
                    nc.vector.tensor_copy(dst, src)
                    n += 1
                    # store A: no w-flip
                    nc.gpsimd.dma_start(
                        ov[b, :, j * CH : (j + 1) * CH], og[:], cond=c_a
                    )
                    # store B: w-flip via reversed w-block read of O
                    og_wrev = og[:].rearrange("p (w h) -> p w h", h=H)[:, ::-1, :]
                    nc.gpsimd.dma_start(
                        ov[b, :, jb * CH : (jb + 1) * CH], og_wrev, cond=c_b
                    )
    nc.compile()
    return nc


_NC_CACHE = None


def _get_program():
    global _NC_CACHE
    if _NC_CACHE is None:
        _NC_CACHE = build_program()
    return _NC_CACHE


def kernel(x, x_flag, y_flag, _trace=False, **trace_kwargs):
    x = np.asarray(x)
    if x.dtype != np.float32:
        x = x.astype(np.float32)
    x_flag = np.asarray(x_flag, dtype=np.float32)
    y_flag = np.asarray(y_flag, dtype=np.float32)
    n = x.shape[0]
    assert n == FULL_B, x.shape

    # host-side schedule: only active samples do device work; spread them
    # round-robin over cores so the busiest core gets ceil(k / n_cores)
    active = [int(i) for i in np.nonzero(x_flag > 0.5)[0]]
    # (core, slot) -> sample index
    assign = {}
    for i, idx in enumerate(active):
        assign[(i % N_CORES, i // N_CORES)] = idx
    assert len(active) <= N_CORES * B

    sample_shape = x.shape[1:]
    dummy = np.zeros((1,) + sample_shape, dtype=np.float32)
    in_maps = []
    for c in range(N_CORES):
        xs, xfs, yfs = [], [], []
        for s in range(B):
            idx = assign.get((c, s))
            if idx is None:
                xs.append(dummy[0])
                xfs.append(0.0)
                yfs.append(0.0)
            else:
                xs.append(x[idx])
                xfs.append(float(x_flag[idx]))
                yfs.append(float(y_flag[idx]))
        in_maps.append(
            {
                "x": np.stack(xs),
                "x_flag": np.array(xfs, dtype=np.float32),
                "y_flag": np.array(yfs, dtype=np.float32),
            }
        )

    nc = _get_program()
    res = run_bass_kernel_spmd(
        nc, in_maps, core_ids=list(range(N_CORES)), trace=_trace, **trace_kwargs
    )

    out = np.zeros((FULL_B,) + sample_shape, dtype=np.float16)
    for (c, s), idx in assign.items():
        out[idx] = res.results[c]["out"][s]
    if _trace:
        return out, res
    return out
